# revision 23
# baseline (speedup 1.0000x reference)
"""Trainium2 Bass kernel for nn_AttEncoder: 2-block causal transformer encoder.

Sharding: data-parallel over batch (B=8) across 8 NeuronCores; each core runs
one full sequence (S=2048, D=128, H=4 heads, L=2 blocks).

Per-core design:
  - activations natural [s-part, d-free] (16 blocks packed into [128, 2048]);
    x^T views produced by PE transposes where matmuls need them
  - attention scores computed transposed S^T[k, q] (lhsT=K^T slice, rhs=Q^T
    slice) with head-pairs row-tiled (tile_position=(32h,0)) into
    double-buffered 2-bank PSUM tiles so ACT exp(kblk i) overlaps PE
    scores(kblk i+1)
  - softmax without max-subtraction (scores are tiny: |s*scale| << 1):
    P^T = exp(scale*S^T), one ACT pass per (kblk, head-pair); causal masking
    via an off-critical-path gpsimd memset (left strip) + triangular mask
    multiply (diagonal block only)
  - P^T @ [V_h | ones] via 64-wide col-tiled matmuls (tile_position=(0,64t))
    accumulate attn^T rows AND softmax denominators Z in one instruction
    stream; transpose back to natural, then normalize (reciprocal+multiply)
    and add the q_in residual in strided sweeps
  - LayerNorm stats per 512-column chunk so chunks pipeline across phases:
    layer-input LN stats via PE (ones-matmuls on x^T/(x^T)^2, sharing lhsT
    loads with the V projection); post-attention LN via bn_stats/bn_aggr;
    rstd via DVE reciprocal + rsqrt-Newton (ACT never leaves the exp table
    set -> exactly one activation-table load)
  - LN affines / qkv biases folded into projection weights on the host; the
    end-of-layer keep mask folded into the LN2 rstd (valid because b1=b2=0
    makes FFN(0)=0); generic inputs fall back to flag-gated slow paths
"""

import os
import numpy as np
import ml_dtypes

B, S, D, H, L = 8, 2048, 128, 4, 2
DK = D // H
SCALE = 1.0 / float(DK) ** 0.5
EPS = 1e-8
P = 128
NB = S // P          # 16 s-blocks
CH = 512             # q-chunk width
NCH = S // CH        # 4 q-chunks
NBC = CH // P        # 4 blocks per chunk
NCORES = 8

_cache = {}


def _build_program(flags):
    from contextlib import ExitStack
    import concourse.bass as bass
    import concourse.tile as tile
    from concourse import bacc, mybir

    f32 = mybir.dt.float32
    bf16 = mybir.dt.bfloat16
    AF = mybir.ActivationFunctionType
    OP = mybir.AluOpType

    aff = flags["affine"]
    has_bv = flags["bv"]
    zb12 = flags["zb12"]

    nc = bacc.Bacc("TRN2", target_bir_lowering=False, debug=False,
                   enable_asserts=False, num_devices=NCORES)

    def din(name, shape, dt):
        return nc.dram_tensor(name, shape, dt, kind="ExternalInput").ap()

    d_x0n = din("x0n", [P, S], bf16)
    d_x0t = din("x0t", [P, S], bf16)
    d_keepn = din("keepn", [P, NB], f32)
    need_keepw = aff or not zb12
    if need_keepw:
        d_keepw = din("keepw", [P, S], f32)
    d_wq = din("wq", [L, P, D], bf16)
    d_wk = din("wk", [L, P, D], bf16)
    d_wv = din("wv", [L, P, D], bf16)
    d_w1 = din("w1", [L, P, D], bf16)
    d_w2 = din("w2", [L, P, D], bf16)
    d_bias = din("bias", [L, P, 4], f32)     # cols: bq, bk, b1, b2
    d_ones = din("ones", [P, DK], bf16)
    d_ident = din("ident", [P, P], f32)
    d_tri = din("tri", [P, H * P], bf16)
    if aff:
        d_gb = din("gb", [2 * L + 1, 2, P, S], f32)
    if has_bv:
        d_bvt = din("bvt", [L, P, D], f32)
    d_out = nc.dram_tensor("out", [P, S], f32, kind="ExternalOutput").ap()

    with tile.TileContext(nc) as tc:
        with ExitStack() as ctx:
            const = ctx.enter_context(tc.tile_pool(name="const", bufs=1))
            acts = ctx.enter_context(tc.tile_pool(name="acts", bufs=1))
            small = ctx.enter_context(tc.tile_pool(name="small", bufs=4))

            def cload(dram_ap, shape, dt, nm):
                t = const.tile(shape, dt, tag=nm, name=nm)
                nc.sync.dma_start(t[:], dram_ap)
                return t

            Wq = [cload(d_wq[l], [P, D], bf16, f"wq{l}") for l in range(L)]
            Wk = [cload(d_wk[l], [P, D], bf16, f"wk{l}") for l in range(L)]
            Wv = [cload(d_wv[l], [P, D], bf16, f"wv{l}") for l in range(L)]
            W1 = [cload(d_w1[l], [P, D], bf16, f"w1{l}") for l in range(L)]
            W2 = [cload(d_w2[l], [P, D], bf16, f"w2{l}") for l in range(L)]
            Bias = [cload(d_bias[l], [P, 4], f32, f"bias{l}") for l in range(L)]
            ones_sb = cload(d_ones, [P, DK], bf16, "ones")
            id_sb = cload(d_ident, [P, P], f32, "ident")
            tri_sb = cload(d_tri, [P, H * P], bf16, "tri")
            keepn_sb = cload(d_keepn, [P, NB], f32, "keepn")
            if need_keepw:
                keepw_sb = cload(d_keepw, [P, S], f32, "keepw")
            if has_bv:
                BV = [cload(d_bvt[l], [P, D], f32, f"bv{l}") for l in range(L)]

            XT0 = const.tile([P, S], bf16, tag="x0t", name="XT0")
            nc.sync.dma_start(XT0[:], d_x0t)
            triv = tri_sb[:].rearrange("p (h c) -> p h c", h=H)

            def transpose_chunk(dst_sb, src_sb, c, pool, ptag, dst_c=None):
                """Transpose blocks 4c..4c+3 of natural<->transposed src into
                dst chunk (via one [P,CH] psum tile)."""
                if dst_c is None:
                    dst_c = c
                pt = pool.tile([P, CH], f32, tag=ptag, name=f"{ptag}tp{c}")
                for bi in range(NBC):
                    blk = c * NBC + bi
                    nc.tensor.transpose(
                        pt[:, bi * P:(bi + 1) * P],
                        src_sb[:, blk * P:(blk + 1) * P],
                        id_sb[:])
                nc.vector.tensor_copy(
                    dst_sb[:, dst_c * CH:(dst_c + 1) * CH], pt[:])

            def rstd_from_var(var_ap, nm, w):
                """r = 1/sqrt(var) on DVE: reciprocal + rsqrt-Newton (no ACT
                table switches). var must include eps."""
                s = small.tile([P, w], f32, tag=f"s{w}", name=f"s{nm}")
                nc.vector.tensor_scalar(s[:], var_ap, 0.5, 0.5, OP.mult, OP.add)
                r = small.tile([P, w], f32, tag=f"r{w}", name=f"r{nm}")
                nc.vector.reciprocal(r[:], s[:])
                t = small.tile([P, w], f32, tag=f"t{w}", name=f"t{nm}")
                for _ in range(2):
                    nc.vector.tensor_mul(t[:], r[:], r[:])
                    nc.vector.tensor_mul(t[:], t[:], var_ap)
                    nc.vector.tensor_scalar(t[:], t[:], -0.5, 1.5,
                                            OP.mult, OP.add)
                    nc.vector.tensor_mul(r[:], r[:], t[:])
                return r

            def ln_smallops_chunk(sums_ap, sums2_ap, nm):
                """Per-chunk: sums/sumsq [P, NBC] -> (m, r) [P, NBC]."""
                m = small.tile([P, NBC], f32, tag="m4", name=f"m{nm}")
                nc.vector.tensor_scalar(m[:], sums_ap, 1.0 / D, None, OP.mult)
                msq = small.tile([P, NBC], f32, tag="msq4", name=f"msq{nm}")
                nc.vector.tensor_mul(msq[:], m[:], m[:])
                var = small.tile([P, NBC], f32, tag="var4", name=f"var{nm}")
                nc.vector.tensor_scalar(var[:], sums2_ap, 1.0 / D, EPS,
                                        OP.mult, OP.add)
                nc.vector.tensor_sub(var[:], var[:], msq[:])
                return m, rstd_from_var(var[:], nm, NBC)

            def ln_bnstats_chunk(Xsrc, c, nm):
                """Per-chunk bn_stats path -> (m, r) [P, NBC]."""
                st6 = small.tile([P, NBC, 6], f32, tag="st6", name=f"st6{nm}")
                for bi in range(NBC):
                    blk = c * NBC + bi
                    nc.vector.bn_stats(st6[:, bi, :],
                                       Xsrc[:, blk * P:(blk + 1) * P])
                mv = small.tile([P, NBC, 2], f32, tag="mv", name=f"mv{nm}")
                for bi in range(NBC):
                    nc.vector.bn_aggr(mv[:, bi, :], st6[:, bi, :])
                m = small.tile([P, NBC], f32, tag="m4", name=f"m{nm}")
                nc.vector.tensor_copy(m[:], mv[:, :, 0])
                var = small.tile([P, NBC], f32, tag="var4", name=f"var{nm}")
                nc.vector.tensor_scalar(var[:], mv[:, :, 1], EPS, None, OP.add)
                return m, rstd_from_var(var[:], nm, NBC)

            def ln_apply_chunk(dst, src, c, m, r):
                for bi in range(NBC):
                    blk = c * NBC + bi
                    nc.vector.tensor_scalar(
                        dst[:, blk * P:(blk + 1) * P],
                        src[:, blk * P:(blk + 1) * P],
                        m[:, bi:bi + 1], r[:, bi:bi + 1],
                        OP.subtract, OP.mult)

            def affine_apply(dst, src, ln_idx):
                gbt = const.tile([2, P, S], f32, tag="gbt", name=f"gbt{ln_idx}",
                                 bufs=1)
                nc.sync.dma_start(gbt[:], d_gb[ln_idx])
                nc.vector.tensor_mul(dst[:], src[:], gbt[0])
                nc.vector.tensor_add(dst[:], dst[:], gbt[1])

            REPEAT = flags.get("repeat", 1)
            for rep in range(REPEAT):
              X = acts.tile([P, S], bf16, tag="Xin", name=f"Xin{rep}")
              nc.sync.dma_start(X[:], d_x0n)
              XT_cur = XT0
              for l in range(L):
                # ------- Phase A: LN1 (PE stats) + projections, per chunk ----
                with tc.tile_pool(name=f"pstat{rep}_{l}", bufs=1,
                                  space="PSUM") as pstat, \
                     tc.tile_pool(name=f"pA{rep}_{l}", bufs=4,
                                  space="PSUM") as pA:
                    X2T = acts.tile([P, S], bf16, tag="X2T", name=f"X2T_{l}")
                    nc.vector.tensor_mul(X2T[:], XT_cur[:], XT_cur[:])

                    sums = pstat.tile([P, NB], f32, tag="sums",
                                      name=f"sums{l}")
                    sums2 = pstat.tile([P, NB], f32, tag="sums2",
                                       name=f"sums2{l}")
                    # Vaug: per kblk 256 cols = [V_h0|1|V_h1|1|V_h2|1|V_h3|1]
                    # (each 32 wide); ones columns let one M=64 matmul produce
                    # both attn^T rows and Z (denominator) rows.
                    Vaug = acts.tile([P, NB * 256], bf16, tag="Vaug",
                                     name=f"Vaug_{l}")
                    vgv = Vaug[:].rearrange("p (i h g dk) -> p i h g dk",
                                            i=NB, h=H, g=2)
                    nc.gpsimd.memset(vgv[:, :, :, 1, :], 1.0)
                    qin = acts.tile([P, S], f32, tag="qin", name=f"qin_{l}")
                    qinT = acts.tile([P, S], bf16, tag="qinT",
                                     name=f"qinT_{l}")
                    QT = acts.tile([P, S], bf16, tag="QT", name=f"QT_{l}")
                    KT = acts.tile([P, S], bf16, tag="KT", name=f"KT_{l}")
                    if aff:
                        qres = acts.tile([P, S], f32, tag="qres",
                                         name=f"qres_{l}")
                    else:
                        qres = qin

                    for c in range(NCH):
                        vps = pA.tile([P, CH], f32, tag="pa",
                                      name=f"vps{l}_{c}")
                        for bi in range(NBC):
                            i = c * NBC + bi
                            nc.tensor.matmul(
                                sums[:, i:i + 1],
                                lhsT=XT_cur[:, i * P:(i + 1) * P],
                                rhs=ones_sb[:, 0:1], start=True, stop=True)
                            nc.tensor.matmul(
                                sums2[:, i:i + 1],
                                lhsT=X2T[:, i * P:(i + 1) * P],
                                rhs=ones_sb[:, 0:1], start=True, stop=True)
                            nc.tensor.matmul(
                                vps[:, bi * P:(bi + 1) * P],
                                lhsT=XT_cur[:, i * P:(i + 1) * P],
                                rhs=Wv[l][:], start=True, stop=True)
                        if has_bv:
                            bvv = BV[l][:].rearrange("p (h dk) -> p h dk", h=H)
                            for bi in range(NBC):
                                vv = vps[:].rearrange(
                                    "p (bi h dk) -> p bi h dk", bi=NBC, h=H)
                                nc.vector.tensor_add(vv[:, bi], vv[:, bi], bvv)
                        nc.vector.tensor_copy(
                            vgv[:, c * NBC:(c + 1) * NBC, :, 0, :],
                            vps[:].rearrange("p (bi h dk) -> p bi h dk",
                                             bi=NBC, h=H))
                        m1, r1 = ln_smallops_chunk(
                            sums[:, c * NBC:(c + 1) * NBC],
                            sums2[:, c * NBC:(c + 1) * NBC], f"a{l}_{c}")
                        ln_apply_chunk(qin, X, c, m1, r1)
                        transpose_chunk(qinT, qin, c, pA, "pa")
                        qp = pA.tile([P, CH], f32, tag="pa", name=f"qp{l}_{c}")
                        nc.tensor.matmul(qp[:], lhsT=Wq[l][:],
                                         rhs=qinT[:, c * CH:(c + 1) * CH],
                                         start=True, stop=True)
                        nc.vector.tensor_scalar(
                            QT[:, c * CH:(c + 1) * CH], qp[:],
                            Bias[l][:, 0:1], None, OP.add)
                        kp = pA.tile([P, CH], f32, tag="pa", name=f"kp{l}_{c}")
                        nc.tensor.matmul(kp[:], lhsT=Wk[l][:],
                                         rhs=XT_cur[:, c * CH:(c + 1) * CH],
                                         start=True, stop=True)
                        nc.vector.tensor_scalar(
                            KT[:, c * CH:(c + 1) * CH], kp[:],
                            Bias[l][:, 1:2], None, OP.add)
                    if aff:
                        affine_apply(qres, qin, 2 * l)

                # ------- Phase B: attention ---------------------------------
                Xnew = acts.tile([P, S], f32, tag="X", name=f"Xnew_{l}",
                                 bufs=2)
                with tc.tile_pool(name=f"psc{rep}_{l}", bufs=2,
                                  space="PSUM") as psc, \
                     tc.tile_pool(name=f"pat{rep}_{l}", bufs=1,
                                  space="PSUM") as pat, \
                     tc.tile_pool(name=f"ptr{rep}_{l}", bufs=1,
                                  space="PSUM") as ptr, \
                     tc.tile_pool(name=f"ptp{rep}_{l}", bufs=4) as ptp:
                    for j in range(NCH):
                        nkb = NBC * j + NBC
                        # attnz: pass p bank holds rows
                        # [attn_{2p}(32) | Z_{2p}(32) | attn_{2p+1} | Z_{2p+1}]
                        attnz = pat.tile([P, 2 * CH], f32, tag="attnz",
                                         name=f"attnz{l}_{j}")
                        for i in range(nkb):
                            r_rel = i - NBC * j
                            qlo = P * r_rel if r_rel >= 0 else 0
                            PT = ptp.tile([P, H, CH], bf16, tag="pt",
                                          name=f"pt{l}_{j}_{i}")
                            if qlo > 0:
                                nc.gpsimd.memset(PT[:, :, :qlo], 0.0)
                            for hp in range(2):
                                scp = psc.tile([P, 2 * CH], f32, tag="scp",
                                               name=f"scp{l}_{j}_{i}_{hp}")
                                for hh in range(2):
                                    h = 2 * hp + hh
                                    nc.tensor.matmul(
                                        scp[:, hh * CH:(hh + 1) * CH],
                                        lhsT=KT[32 * h:32 * (h + 1),
                                                i * P:(i + 1) * P],
                                        rhs=QT[32 * h:32 * (h + 1),
                                               j * CH:(j + 1) * CH],
                                        tile_position=(32 * h, 0),
                                        start=True, stop=True)
                                scv = scp[:].rearrange("p (h q) -> p h q", h=2)
                                nc.scalar.activation(
                                    PT[:, 2 * hp:2 * hp + 2, qlo:],
                                    scv[:, :, qlo:],
                                    AF.Exp, scale=SCALE)
                                if r_rel >= 0:
                                    nc.vector.tensor_mul(
                                        PT[:, 2 * hp:2 * hp + 2,
                                           qlo:qlo + P],
                                        PT[:, 2 * hp:2 * hp + 2,
                                           qlo:qlo + P],
                                        triv[:, 2 * hp:2 * hp + 2, :])
                            for h in range(H):
                                p_, t_ = divmod(h, 2)
                                nc.tensor.matmul(
                                    attnz[64 * t_:64 * (t_ + 1),
                                          p_ * CH:(p_ + 1) * CH],
                                    lhsT=Vaug[:, 256 * i + 64 * h:
                                              256 * i + 64 * (h + 1)],
                                    rhs=PT[:, h, :],
                                    tile_position=(0, 64 * t_),
                                    start=(i == 0), stop=(i == nkb - 1))
                        # evict: copy to SBUF, transpose to natural, then
                        # normalize by Z and add the residual in one sweep
                        atz = small.tile([P, 2 * CH], f32, tag="atz",
                                         name=f"atz{l}_{j}")
                        nc.vector.tensor_copy(atz[:], attnz[:])
                        pt2 = ptr.tile([P, 2 * CH], f32, tag="pt2",
                                       name=f"pt2{l}_{j}")
                        for bb in range(2 * NBC):
                            nc.tensor.transpose(
                                pt2[:, bb * P:(bb + 1) * P],
                                atz[:, bb * P:(bb + 1) * P], id_sb[:])
                        # pt2 cols: ps(2) x bi(4) x [a|z|a|z] (hh x kind x 32)
                        tv = pt2[:].rearrange(
                            "p (ps bi hh kind dk) -> p ps bi hh kind dk",
                            ps=2, bi=NBC, hh=2, kind=2)
                        zi = small.tile([P, 2, NBC, 2, 32], f32, tag="zi",
                                        name=f"zi{l}_{j}")
                        nc.vector.reciprocal(zi[:], tv[:, :, :, :, 1, :])
                        anorm = small.tile([P, 2, NBC, 2, 32], f32,
                                           tag="anorm", name=f"anorm{l}_{j}")
                        nc.vector.tensor_mul(anorm[:], tv[:, :, :, :, 0, :],
                                             zi[:])
                        # Xnew/qres chunk view: cols bi*128 + ps*64 + hh*32 + dk
                        xv = Xnew[:, j * CH:(j + 1) * CH].rearrange(
                            "p (bi ps hh dk) -> p ps bi hh dk",
                            bi=NBC, ps=2, hh=2)
                        qv = qres[:, j * CH:(j + 1) * CH].rearrange(
                            "p (bi ps hh dk) -> p ps bi hh dk",
                            bi=NBC, ps=2, hh=2)
                        nc.vector.tensor_add(xv, anorm[:], qv)

                # ------- Phase C: LN2 + FFN, per chunk -----------------------
                with tc.tile_pool(name=f"pC{rep}_{l}", bufs=2,
                                  space="PSUM") as pC:
                    z2 = acts.tile([P, S], f32, tag="z2", name=f"z2_{l}")
                    z2T = acts.tile([P, S], bf16, tag="z2T", name=f"z2T_{l}")
                    Xout = acts.tile([P, S], f32, tag="Xout", name=f"Xout_{l}",
                                     bufs=2)
                    if aff:
                        z2res = acts.tile([P, S], f32, tag="z2res",
                                          name=f"z2res_{l}")
                    else:
                        z2res = z2
                    if l + 1 < L:
                        XTn = acts.tile([P, S], bf16, tag="XT",
                                        name=f"XT_{l + 1}")
                    for c in range(NCH):
                        m2, r2 = ln_bnstats_chunk(Xnew, c, f"b{l}_{c}")
                        if zb12:
                            # b1 == b2 == 0: FFN(0-row) == 0, so folding keep
                            # into the LN2 rstd zeroes masked rows end-to-end
                            r2k = small.tile([P, NBC], f32, tag="r2k",
                                             name=f"r2k{l}_{c}")
                            nc.vector.tensor_mul(
                                r2k[:], r2[:],
                                keepn_sb[:, c * NBC:(c + 1) * NBC])
                            r2 = r2k
                        ln_apply_chunk(z2, Xnew, c, m2, r2)
                        if aff:
                            pass  # z2res filled after loop
                        transpose_chunk(z2T, z2, c, pC, "pc")
                        hp_ps = pC.tile([P, CH], f32, tag="hp",
                                        name=f"hp{l}_{c}")
                        nc.tensor.matmul(hp_ps[:], lhsT=W1[l][:],
                                         rhs=z2T[:, c * CH:(c + 1) * CH],
                                         start=True, stop=True)
                        Hb = acts.tile([P, CH], bf16, tag="Hb",
                                       name=f"Hb{l}_{c}", bufs=2)
                        nc.vector.tensor_scalar(
                            Hb[:], hp_ps[:], Bias[l][:, 2:3], 0.0,
                            OP.add, OP.max)
                        o2p = pC.tile([P, CH], f32, tag="o2p",
                                      name=f"o2p{l}_{c}")
                        nc.tensor.matmul(o2p[:], lhsT=W2[l][:], rhs=Hb[:],
                                         start=True, stop=True)
                        o2s = small.tile([P, CH], f32, tag="o2s",
                                         name=f"o2s{l}_{c}")
                        nc.vector.tensor_scalar(
                            o2s[:], o2p[:], Bias[l][:, 3:4], None, OP.add)
                        po = pC.tile([P, CH], f32, tag="pc", name=f"po{l}_{c}")
                        for bi in range(NBC):
                            nc.tensor.transpose(
                                po[:, bi * P:(bi + 1) * P],
                                o2s[:, bi * P:(bi + 1) * P], id_sb[:])
                        if aff:
                            # z2res = z2*G+B must exist before residual; for
                            # the (rare) affine path keep it simple: compute
                            # per chunk with G/B slices
                            gbt = const.tile([2, P, S], f32, tag="gbt",
                                             name=f"gbt{2*l+1}", bufs=1)
                            if c == 0:
                                nc.sync.dma_start(gbt[:], d_gb[2 * l + 1])
                            sl = slice(c * CH, (c + 1) * CH)
                            nc.vector.tensor_mul(z2res[:, sl], z2[:, sl],
                                                 gbt[0][:, sl])
                            nc.vector.tensor_add(z2res[:, sl], z2res[:, sl],
                                                 gbt[1][:, sl])
                        nc.vector.tensor_add(
                            Xout[:, c * CH:(c + 1) * CH], po[:],
                            z2res[:, c * CH:(c + 1) * CH])
                        if not zb12:
                            nc.vector.tensor_mul(
                                Xout[:, c * CH:(c + 1) * CH],
                                Xout[:, c * CH:(c + 1) * CH],
                                keepw_sb[:, c * CH:(c + 1) * CH])
                        if l + 1 < L:
                            transpose_chunk(XTn, Xout, c, pC, "pc")
                    X = Xout
                    if l + 1 < L:
                        XT_cur = XTn

              # ------- Final LN, per chunk ----------------------------------
              with tc.tile_pool(name=f"pF{rep}", bufs=2, space="PSUM") as pF:
                  OUTt = acts.tile([P, S], f32, tag="OUT", name="OUT")
                  if aff:
                      gbt = const.tile([2, P, S], f32, tag="gbt",
                                       name=f"gbt{2*L}", bufs=1)
                      nc.sync.dma_start(gbt[:], d_gb[2 * L])
                  for c in range(NCH):
                      mf, rf = ln_bnstats_chunk(X, c, f"f{c}")
                      if aff:
                          zf = acts.tile([P, CH], f32, tag="zf",
                                         name=f"zf{c}", bufs=2)
                          for bi in range(NBC):
                              blk = c * NBC + bi
                              nc.vector.tensor_scalar(
                                  zf[:, bi * P:(bi + 1) * P],
                                  X[:, blk * P:(blk + 1) * P],
                                  mf[:, bi:bi + 1], rf[:, bi:bi + 1],
                                  OP.subtract, OP.mult)
                          sl = slice(c * CH, (c + 1) * CH)
                          nc.vector.tensor_mul(OUTt[:, sl], zf[:], gbt[0][:, sl])
                          nc.vector.tensor_add(OUTt[:, sl], OUTt[:, sl],
                                               gbt[1][:, sl])
                          nc.vector.tensor_mul(OUTt[:, sl], OUTt[:, sl],
                                               keepw_sb[:, sl])
                      else:
                          rk = small.tile([P, NBC], f32, tag="rk",
                                          name=f"rk{c}")
                          nc.vector.tensor_mul(
                              rk[:], rf[:],
                              keepn_sb[:, c * NBC:(c + 1) * NBC])
                          ln_apply_chunk(OUTt, X, c, mf, rk)
                      nc.sync.dma_start(d_out[:, c * CH:(c + 1) * CH],
                                        OUTt[:, c * CH:(c + 1) * CH])

    nc.compile()
    return nc


def _get_program(flags):
    key = tuple(sorted(flags.items()))
    if key not in _cache:
        _cache[key] = _build_program(flags)
    return _cache[key]


def _prep_inputs(log_seqs, seqs, Wqkv, bqkv, ln1_g, ln1_b, ln2_g, ln2_b,
                 W1, b1, W2, b2, lng, lnb):
    bf = ml_dtypes.bfloat16
    f32 = np.float32
    log_seqs = np.asarray(log_seqs)
    seqs = np.asarray(seqs, dtype=f32)
    Wqkv = np.asarray(Wqkv, dtype=f32)
    bqkv = np.asarray(bqkv, dtype=f32)
    ln1_g = np.asarray(ln1_g, dtype=f32); ln1_b = np.asarray(ln1_b, dtype=f32)
    ln2_g = np.asarray(ln2_g, dtype=f32); ln2_b = np.asarray(ln2_b, dtype=f32)
    W1 = np.asarray(W1, dtype=f32); b1 = np.asarray(b1, dtype=f32)
    W2 = np.asarray(W2, dtype=f32); b2 = np.asarray(b2, dtype=f32)
    lng = np.asarray(lng, dtype=f32); lnb = np.asarray(lnb, dtype=f32)

    trivial_aff = (np.all(ln1_g == 1) and np.all(ln1_b == 0)
                   and np.all(ln2_g == 1) and np.all(ln2_b == 0)
                   and np.all(lng == 1) and np.all(lnb == 0))
    has_bv = bool(np.any(bqkv[:, 2] != 0))
    flags = {"affine": not trivial_aff, "bv": has_bv,
             "zb12": bool(np.all(b1 == 0) and np.all(b2 == 0)),
             "repeat": int(os.environ.get("KERNEL_REPEAT", "1"))}

    # Effective weights: fold LN affine into the consuming projection.
    wq_eff = np.empty((L, P, D), f32); bq_eff = np.empty((L, P), f32)
    w1_eff = np.empty((L, P, D), f32); b1_eff = np.empty((L, P), f32)
    for l in range(L):
        wq_eff[l] = ln1_g[l][:, None] * Wqkv[l, 0]
        bq_eff[l] = ln1_b[l] @ Wqkv[l, 0] + bqkv[l, 0]
        w1_eff[l] = ln2_g[l][:, None] * W1[l]
        b1_eff[l] = ln2_b[l] @ W1[l] + b1[l]

    shared = {
        "wq": wq_eff.astype(bf),
        "wk": Wqkv[:, 1].astype(bf),
        "wv": Wqkv[:, 2].astype(bf),
        "w1": w1_eff.astype(bf),
        "w2": W2.astype(bf),
        "bias": np.stack([np.stack([bq_eff[l], bqkv[l, 1], b1_eff[l], b2[l]],
                                   axis=1) for l in range(L)]).astype(f32),
        "ones": np.ones((P, DK), bf),
        "ident": np.eye(P, dtype=f32),
        "tri": np.tile(np.triu(np.ones((P, P), f32)), (1, H)).astype(bf),
    }
    if flags["affine"]:
        def nat_tile(v):
            t = np.broadcast_to(v[None, :], (S, D))
            return np.ascontiguousarray(
                t.reshape(NB, P, D).transpose(1, 0, 2).reshape(P, S))
        gbs = []
        for l in range(L):
            gbs.append(np.stack([nat_tile(ln1_g[l]), nat_tile(ln1_b[l])]))
            gbs.append(np.stack([nat_tile(ln2_g[l]), nat_tile(ln2_b[l])]))
        gbs.append(np.stack([nat_tile(lng), nat_tile(lnb)]))
        shared["gb"] = np.stack(gbs).astype(f32)
    if flags["bv"]:
        shared["bvt"] = np.broadcast_to(
            bqkv[:, 2][:, None, :], (L, P, D)).astype(f32).copy()

    in_maps = []
    for b in range(B):
        keep = (log_seqs[b] != 0).astype(f32)
        x0 = seqs[b] * keep[:, None]
        x0n = np.ascontiguousarray(
            x0.reshape(NB, P, D).transpose(1, 0, 2).reshape(P, S))
        keepn = np.ascontiguousarray(keep.reshape(NB, P).T)
        keepw = np.ascontiguousarray(
            np.broadcast_to(keepn[:, :, None], (P, NB, P)).reshape(P, S))
        m = dict(shared)
        m["x0n"] = x0n.astype(bf)
        m["x0t"] = np.ascontiguousarray(x0.T).astype(bf)
        m["keepn"] = keepn.astype(f32)
        if flags["affine"] or not flags["zb12"]:
            m["keepw"] = keepw.astype(f32)
        in_maps.append(m)
    return flags, in_maps


def kernel(**inputs):
    from concourse import bass_utils
    flags, in_maps = _prep_inputs(**inputs)
    nc = _get_program(flags)
    trace = bool(int(os.environ.get("KERNEL_TRACE", "0")))
    res = bass_utils.run_bass_kernel_spmd(
        nc, in_maps, core_ids=list(range(NCORES)), trace=trace)
    kernel.last_result = res
    outs = []
    for b in range(B):
        o = res.results[b]["out"]
        outs.append(o.reshape(P, NB, P).transpose(1, 0, 2).reshape(S, D))
    return np.stack(outs).astype(np.float32)


kernel.last_result = None
